# revision 5
# baseline (speedup 1.0000x reference)
"""Trainium2 Bass kernel for nn_ECA_69544110457542.

Math (per row r=(b,t)):
  dyn   = x[:, :31] @ Wd + bd
  value = x[:, 31] * Wv[0] + bv
  xhn   = [dyn | human@Wh+bh | nature@Wn+bn]                      (768 ch)
  pre_j = sum_k cw[t,k] * xhn[perm[ainv[j]+k-3]] + conv_b[t]      (j<256)
  sel   = softmax(relu(pre))
  out   = 0.5*(dyn*sel) @ Wvd1  +  0.5*dyn @ Wvd1 + value @ Wvd2 + bvd
          `------ device ------'  `------- folded into wdf (host) -----'

Per-tile structure (16 tiles of 128 rows per core):
  PE    : GA = act @ WgA (k0,k1 cols, 2 K-splits), GB = act @ WgB (k2..k4),
          pdf[0:512] = act @ [wdyn | wfold] (one bf16 matmul), and the
          deferred z-path of the previous tile (zT @ 0.5*Wvd1 accumulated
          into pdf[256:512]).
  ACT   : c0/c1 = GA k-slices drained with scale=cw[t,k] (conv weights are
          free inside the drain), exp(relu) with accum_out row-sum, and the
          final out drain of the previous tile.
  DVE   : c0+c1, then 3 chained scalar_tensor_tensor ops reading GB's
          k-slices straight from PSUM, relu, reciprocal, softmax gate.
  DMA   : the z transpose runs on the DMA XBAR (dma_start_transpose), not
          on PE; outputs stream out per tile; all plain DMAs are issued
          from the Pool sequencer (SP DMA config costs 565ns, Pool ~25ns).

PSUM: GA [P,512] x2 + GB [P,768] x2 + pdf [P,512] x2 = exactly 8 banks.
SBUF working tiles use bufs=16 (no reuse) so no instruction ever needs a
second (WAR) semaphore wait; this build rejects multi-wait instructions.

Sharding: pure data parallel, 32 batches per core on 8 cores.
"""

import sys

sys.path.insert(0, "/opt/trn_rl_repo")

from contextlib import ExitStack

import ml_dtypes
import numpy as np

import concourse.bacc as bacc
import concourse.bass as bass
import concourse.tile as tile
from concourse import mybir
from concourse.bass_utils import run_bass_kernel_spmd

# problem constants
B, T, E = 256, 64, 256
XS, DS = 32, 31
HT, NT_ = 80, 80
C = 3 * E
KW = 5
NCORES = 8
BPC = B // NCORES          # 32 batches per core
R = BPC * T                # 2048 rows per core
P = 128
NTILES = R // P            # 16
AK = XS + 1 + HT + NT_     # 193 act rows: x(32) | ones | h(80) | n(80)
K2 = AK - 128              # 65
NG = KW * E                # 1280 gathered columns

# packed-constants layout, fp32 slot offsets in [128, WPACK]
O_WGA1 = 0                  # bf16 [128, 512]  (k0,k1 cols, K rows 0:128)
O_WGA2 = 256                # bf16 [65, 512]   (k0,k1 cols, K rows 128:193)
O_WGB1 = 512                # bf16 [128, 768]  (k2..k4)
O_WGB2 = 896                # bf16 [65, 768]
O_WDF = 1280                # bf16 [128, 512]: wdyn | wfold (rows 33: zero)
O_WV1 = 1536                # bf16 [128, 512]: 0.5*Wvd1 rows 0:128 | 128:256
O_CW = 1792                 # fp32 [128, 5]
O_CB = 1797                 # fp32 [128, 1]
WPACK = 1798

F32 = mybir.dt.float32
BF16 = mybir.dt.bfloat16
MULT = mybir.AluOpType.mult
ADD = mybir.AluOpType.add
IDENT = mybir.ActivationFunctionType.Identity
EXP = mybir.ActivationFunctionType.Exp

_NC_CACHE = None
LAST_RESULTS = None
TRACE = False


def _patched_drain_and_barrier(self, tick_clock, wait_clock):
    # The stock kernel-tail drain puts every processor's final-tick wait on a
    # single Drain instruction; this walrus build rejects multi-wait
    # instructions, so spread the waits over a chain of drains instead.
    import bass_rust as _br
    from concourse.vector_clock import ScopedClock

    nc = self.nc
    drain_inst = nc.sync.drain()
    wait_clock.add_sem_waits(
        drain_inst.ins, ScopedClock({None: tick_clock.global_clock})
    )
    si = drain_inst.ins.sync_info
    if si is not None and len(si.on_wait) > 1:
        waits = list(si.on_wait)
        drain_inst.ins.sync_info = _br.SyncInfo(
            on_wait=[waits[0]], on_update=list(si.on_update)
        )
        for w in waits[1:]:
            d2 = nc.sync.drain()
            d2.ins.sync_info = _br.SyncInfo(on_wait=[w], on_update=[])
    nc.all_engine_barrier()
    assert self.sems is not None
    popped = nc._tile_sem_poison_stack.pop()
    assert popped is self._sem_poison
    nc.clear_and_free_semaphores(list(self.sems.allocated().values()))
    nc.all_engine_barrier()


tile.TileContext._drain_and_barrier = _patched_drain_and_barrier


def _build_nc():
    # Bacc (not plain Bass): its finalize() runs move_matmul_waits_to_ldweights
    # and generate_event_semaphores, which split multi-wait instructions into
    # EventSemaphore preludes — this build's ISA allows one wait per
    # instruction (two on EventSemaphore).
    nc = bacc.Bacc()
    actb_d = nc.dram_tensor("actb", [P, 2 * R], BF16, kind="ExternalInput")
    wpack_d = nc.dram_tensor("wpack", [P, WPACK], F32, kind="ExternalInput")
    out_d = nc.dram_tensor("out", [R, E], F32, kind="ExternalOutput")
    actb3 = actb_d[:, :].rearrange("p (two r) -> p two r", two=2)

    with tile.TileContext(nc) as tc, ExitStack() as ctx:
        consts = ctx.enter_context(tc.tile_pool(name="consts", bufs=1))
        psA = ctx.enter_context(tc.tile_pool(name="psA", bufs=2, space="PSUM"))
        psB = ctx.enter_context(tc.tile_pool(name="psB", bufs=2, space="PSUM"))
        psD = ctx.enter_context(tc.tile_pool(name="psD", bufs=2, space="PSUM"))
        pc = ctx.enter_context(tc.tile_pool(name="pc", bufs=16))
        ps = ctx.enter_context(tc.tile_pool(name="ps", bufs=16))
        pz = ctx.enter_context(tc.tile_pool(name="pz", bufs=16))
        pzT = ctx.enter_context(tc.tile_pool(name="pzT", bufs=16))
        po = ctx.enter_context(tc.tile_pool(name="po", bufs=16))
        psm = ctx.enter_context(tc.tile_pool(name="psm", bufs=16))

        wp = consts.tile([P, WPACK], F32)
        # small weights first (first-needed), then the two Wg chunks
        nc.gpsimd.dma_start(wp[:, O_WDF:WPACK], wpack_d[:, O_WDF:WPACK])
        nc.gpsimd.dma_start(wp[:, 0:512], wpack_d[:, 0:512])
        nc.gpsimd.dma_start(wp[:, 512:1280], wpack_d[:, 512:1280])

        wpb = wp[:].bitcast(BF16)
        wgA1 = wpb[:, 2 * O_WGA1 : 2 * O_WGA1 + 512]
        wgA2 = wpb[0:K2, 2 * O_WGA2 : 2 * O_WGA2 + 512]
        wgB1 = wpb[:, 2 * O_WGB1 : 2 * O_WGB1 + 768]
        wgB2 = wpb[0:K2, 2 * O_WGB2 : 2 * O_WGB2 + 768]
        wdf = wpb[:, 2 * O_WDF : 2 * O_WDF + 512]
        wv1a = wpb[:, 2 * O_WV1 : 2 * O_WV1 + E]
        wv1b = wpb[:, 2 * O_WV1 + E : 2 * O_WV1 + 2 * E]
        cw = wp[:, O_CW : O_CW + KW]
        cb = wp[:, O_CB : O_CB + 1]

        # activations: one resident SBUF tile, 16 chunked DMAs (subtile deps
        # give each tile's first reader a precise single wait)
        ab = consts.tile([P, 2, R], BF16)
        for i in range(NTILES):
            nc.gpsimd.dma_start(
                ab[:, :, i * P : (i + 1) * P], actb3[:, :, i * P : (i + 1) * P]
            )

        # warmup: each engine observes the weight DMAs once so later compute
        # instructions never need a second sem wait for them
        nc.tensor.ldweights(wdf[0:1, 0:2])
        nc.tensor.ldweights(wgA1[0:1, 0:2])
        nc.tensor.ldweights(wgB1[0:1, 0:2])
        at = psm.tile([P, 1], F32, tag="wm")
        nc.scalar.copy(at[:], cb)
        dt_ = psm.tile([P, 1], F32, tag="wm2")
        nc.vector.tensor_copy(dt_[:], cb)

        prev = None

        def flush_z(prev):
            # z-path of tile j, emitted one iteration later so PE never
            # waits on the current tile's softmax chain
            j, zT0, zT1, pdfj = prev
            nc.tensor.matmul(
                pdfj[:, 256:512], zT0[:], wv1a,
                start=False, stop=False, skip_group_check=True,
            )
            nc.tensor.matmul(
                pdfj[:, 256:512], zT1[:], wv1b,
                start=False, stop=True, skip_group_check=True,
            )
            ob = po.tile([P, E], F32, tag="ob", name=f"ob{j}")
            nc.scalar.copy(ob[:], pdfj[:, 256:512])
            nc.gpsimd.dma_start(out_d[j * P : (j + 1) * P, :], ob[:])

        for i in range(NTILES):
            rows = slice(i * P, (i + 1) * P)
            b0 = ab[:, 0, rows]
            b1 = ab[0:K2, 1, rows]

            # absorb this tile's actb-chunk DMA wait so the first real
            # matmul's single wait slot is free for its PSUM WAR dep
            nc.tensor.ldweights(ab[0:1, 0, i * P : i * P + 2])

            GA = psA.tile([P, 512], F32, tag="GA")
            GB = psB.tile([P, 768], F32, tag="GB")
            pdf = psD.tile([P, 512], F32, tag="pdf")

            nc.tensor.matmul(GA[:, 0:512], b0, wgA1,
                             start=True, stop=False, skip_group_check=True)
            nc.tensor.matmul(GA[:, 0:512], b1, wgA2,
                             start=False, stop=True, skip_group_check=True)
            nc.tensor.matmul(GB[:, 0:512], b0, wgB1[:, 0:512],
                             start=True, stop=False, skip_group_check=True)
            nc.tensor.matmul(GB[:, 0:512], b1, wgB2[:, 0:512],
                             start=False, stop=True, skip_group_check=True)
            nc.tensor.matmul(GB[:, 512:768], b0, wgB1[:, 512:768],
                             start=True, stop=False, skip_group_check=True)
            nc.tensor.matmul(GB[:, 512:768], b1, wgB2[:, 512:768],
                             start=False, stop=True, skip_group_check=True)
            nc.tensor.matmul(pdf[:, 0:512], b0, wdf,
                             start=True, stop=True, skip_group_check=True)

            if prev is not None:
                flush_z(prev)

            # conv combine: k0/k1 drained by ACT with the per-t conv weight
            # as the activation scale; k2..k4 combined by DVE straight from
            # PSUM via chained scalar_tensor_tensor
            c0 = pc.tile([P, E], BF16, tag="c0")
            nc.scalar.activation(c0[:], GA[:, 0:256], func=IDENT,
                                 scale=cw[:, 0:1], bias=cb)
            c1 = pc.tile([P, E], BF16, tag="c1")
            nc.scalar.activation(c1[:], GA[:, 256:512], func=IDENT,
                                 scale=cw[:, 1:2])
            s1 = ps.tile([P, E], BF16, tag="s1")
            nc.vector.tensor_add(s1[:], c0[:], c1[:])
            s2 = ps.tile([P, E], BF16, tag="s2")
            nc.vector.scalar_tensor_tensor(s2[:], GB[:, 0:256], cw[:, 2:3],
                                           s1[:], op0=MULT, op1=ADD)
            s3 = ps.tile([P, E], BF16, tag="s3")
            nc.vector.scalar_tensor_tensor(s3[:], GB[:, 256:512], cw[:, 3:4],
                                           s2[:], op0=MULT, op1=ADD)
            s4 = ps.tile([P, E], BF16, tag="s4")
            nc.vector.scalar_tensor_tensor(s4[:], GB[:, 512:768], cw[:, 4:5],
                                           s3[:], op0=MULT, op1=ADD)
            rel = ps.tile([P, E], BF16, tag="rel")
            nc.vector.tensor_scalar_max(rel[:], s4[:], 0.0)

            # exp + free row-sum via accum_out
            exm = pc.tile([P, E], BF16, tag="exm")
            ssum = psm.tile([P, 1], F32, tag="ssum")
            nc.scalar.activation(exm[:], rel[:], func=EXP, accum_out=ssum[:])
            sinv = psm.tile([P, 1], F32, tag="sinv")
            nc.vector.reciprocal(sinv[:], ssum[:])

            # z = (exm / S) * dyn  (the 0.5 is folded into Wvd1)
            z = pz.tile([P, E], BF16, tag="z")
            nc.vector.scalar_tensor_tensor(z[:], exm[:], sinv[:],
                                           pdf[:, 0:256], op0=MULT, op1=MULT)

            # transpose z on the DMA XBAR (PE transpose would burn a PSUM
            # bank and an extra drain)
            zT0 = pzT.tile([P, P], BF16, tag="zT0", name=f"zT0_{i}")
            zT1 = pzT.tile([P, P], BF16, tag="zT1", name=f"zT1_{i}")
            nc.sync.dma_start_transpose(zT0[:], z[:, 0:128])
            nc.sync.dma_start_transpose(zT1[:], z[:, 128:256])

            prev = (i, zT0, zT1, pdf)

        flush_z(prev)

    nc.finalize()
    return nc


def _host_prep(x, human, nature, perm, Wv, bv, Wd, bd, Wh, bh, Wn, bn,
               conv_w, conv_b, Wvd, bvd):
    f = np.float32
    bf = ml_dtypes.bfloat16
    x = np.asarray(x, f)
    human = np.asarray(human, f)
    nature = np.asarray(nature, f)
    Wv = np.asarray(Wv, f); bv = np.asarray(bv, f)
    Wd = np.asarray(Wd, f); bd = np.asarray(bd, f)
    Wh = np.asarray(Wh, f); bh = np.asarray(bh, f)
    Wn = np.asarray(Wn, f); bn = np.asarray(bn, f)
    conv_w = np.asarray(conv_w, f)
    conv_b = np.asarray(conv_b, f)
    Wvd = np.asarray(Wvd, f); bvd = np.asarray(bvd, f)
    perm = np.asarray(perm).astype(np.int64)

    Wvd1 = Wvd[:E, :]
    Wvd2 = Wvd[E:, :]

    acts = np.concatenate(
        [
            x.reshape(B * T, XS),
            np.ones((B * T, 1), f),
            human.reshape(B * T, HT),
            nature.reshape(B * T, NT_),
        ],
        axis=1,
    )
    actsT = np.ascontiguousarray(acts.T)  # [193, B*T]
    actb = np.zeros((P, 2, B * T), bf)
    actb[:, 0, :] = actsT[0:128]
    actb[0:K2, 1, :] = actsT[128:AK]

    wpack = np.zeros((P, WPACK), f)
    wpv = wpack.view(bf)  # bf16 alias [128, 2*WPACK]

    # dyn | folded-linear weights (rows 33:128 zero so the matmul can use
    # the full 128-row stationary block)
    wdf = np.zeros((128, 512), f)
    wdf[0:DS, 0:E] = Wd
    wdf[32, 0:E] = bd
    wdf[0:DS, E:512] = 0.5 * (Wd @ Wvd1)
    wdf[31, E:512] = Wv[0] @ Wvd2
    wdf[32, E:512] = 0.5 * (bd @ Wvd1) + bv @ Wvd2 + bvd
    wpv[:, 2 * O_WDF : 2 * O_WDF + 512] = wdf.astype(bf)

    # gathered conv weights (bf16)
    ainv = np.argsort(perm)
    Wg = np.zeros((AK, NG), f)
    for k in range(KW):
        pos = ainv[:E] + k - 3
        for j in range(E):
            pj = pos[j]
            if 0 <= pj < C:
                c = perm[pj]
                col = k * E + j
                if c < E:
                    Wg[0:DS, col] = Wd[:, c]
                    Wg[32, col] = bd[c]
                elif c < 2 * E:
                    Wg[33:113, col] = Wh[:, c - E]
                    Wg[32, col] = bh[c - E]
                else:
                    Wg[113:193, col] = Wn[:, c - 2 * E]
                    Wg[32, col] = bn[c - 2 * E]
    wpv[:, 2 * O_WGA1 : 2 * O_WGA1 + 512] = Wg[0:128, 0:512].astype(bf)
    wpv[0:K2, 2 * O_WGA2 : 2 * O_WGA2 + 512] = Wg[128:AK, 0:512].astype(bf)
    wpv[:, 2 * O_WGB1 : 2 * O_WGB1 + 768] = Wg[0:128, 512:1280].astype(bf)
    wpv[0:K2, 2 * O_WGB2 : 2 * O_WGB2 + 768] = Wg[128:AK, 512:1280].astype(bf)

    # 0.5 * Wvd1 (bf16), split into two K-chunks
    wv1 = (0.5 * Wvd1).astype(bf)
    wpv[:, 2 * O_WV1 : 2 * O_WV1 + E] = wv1[0:128]
    wpv[:, 2 * O_WV1 + E : 2 * O_WV1 + 2 * E] = wv1[128:256]

    wpack[:, O_CW : O_CW + KW] = np.tile(conv_w[:, 0, :], (2, 1))
    wpack[:, O_CB] = np.tile(conv_b, 2)
    return actb, wpack


def kernel(**inputs):
    global _NC_CACHE, LAST_RESULTS
    actb, wpack = _host_prep(**inputs)

    if _NC_CACHE is None:
        _NC_CACHE = _build_nc()
    nc = _NC_CACHE

    in_maps = []
    for ci in range(NCORES):
        sb = np.ascontiguousarray(actb[:, :, ci * R : (ci + 1) * R]).reshape(
            P, 2 * R
        )
        in_maps.append({"actb": sb, "wpack": wpack})

    res = run_bass_kernel_spmd(nc, in_maps, core_ids=list(range(NCORES)), trace=TRACE)
    LAST_RESULTS = res

    out = np.empty((B, T, E), np.float32)
    for ci in range(NCORES):
        out[ci * BPC : (ci + 1) * BPC] = res.results[ci]["out"].reshape(BPC, T, E)
    return out


# revision 7
# speedup vs baseline: 1.8704x; 1.8704x over previous
"""Trainium2 Bass kernel for nn_ECA_69544110457542.

Math (per row r=(b,t)):
  dyn   = x[:, :31] @ Wd + bd
  value = x[:, 31] * Wv[0] + bv
  xhn   = [dyn | human@Wh+bh | nature@Wn+bn]                      (768 ch)
  pre_j = sum_k cw[t,k] * xhn[perm[ainv[j]+k-3]] + conv_b[t]      (j<256)
  sel   = softmax(relu(pre))
  out   = 0.5*(dyn*sel) @ Wvd1  +  0.5*dyn @ Wvd1 + value @ Wvd2 + bvd
          `------ device ------'  `------- folded into wdf (host) -----'

Two-stage software pipeline over 16 row-tiles (128 rows each):
  stage 1 (tile i):   PE G-matmuls (gathered conv columns, 2 K-splits) and
                      the merged bf16 [wdyn|wfold] matmul; ACT drains the
                      k0/k1 PSUM slices with scale=cw[t,k] (conv weights are
                      free in the drain); DVE chains k2..k4 straight out of
                      PSUM via scalar_tensor_tensor, then relu.
  stage 2 (tile i-1): ACT exp(+accum row-sum); DVE reciprocal + softmax
                      gate; PE transposes z and accumulates zT @ 0.5*Wvd1
                      into the folded-output PSUM; ACT drains it; Pool
                      issues the output DMA.
Deferring stage 2 by one iteration keeps every cross-engine wait pointing
at work that is already done, so no engine idles inside a tile.

PSUM (8 banks): T1 k0k1 [P,512]x1, T2 k2k3 [P,512]x1, T3 k4 [P,256]x2,
pdf [P,512]x3, ptz (z transpose) x1.  SBUF working tiles use bufs=16 (no
reuse => no WAR semaphores).  Bacc.finalize() splits any remaining
multi-wait instructions into EventSemaphore preludes (1-wait ISA limit).

Sharding: pure data parallel, 32 batches per core on 8 cores.
"""

import sys

sys.path.insert(0, "/opt/trn_rl_repo")

from contextlib import ExitStack

import ml_dtypes
import numpy as np

import concourse.bacc as bacc
import concourse.bass as bass
import concourse.tile as tile
from concourse import mybir
from concourse.bass_utils import run_bass_kernel_spmd

# problem constants
B, T, E = 256, 64, 256
XS, DS = 32, 31
HT, NT_ = 80, 80
C = 3 * E
KW = 5
NCORES = 8
BPC = B // NCORES          # 32 batches per core
R = BPC * T                # 2048 rows per core
P = 128
NTILES = R // P            # 16
AK = XS + 1 + HT + NT_     # 193 act rows: x(32) | ones | h(80) | n(80)
K2 = AK - 128              # 65
NG = KW * E                # 1280 gathered columns

# packed-constants layout, fp32 slot offsets in [128, WPACK]
O_WGA1 = 0                  # bf16 [128, 512]  (k0,k1 cols, K rows 0:128)
O_WGA2 = 256                # bf16 [65, 512]   (k0,k1 cols, K rows 128:193)
O_WGB1 = 512                # bf16 [128, 768]  (k2..k4)
O_WGB2 = 896                # bf16 [65, 768]
O_WDF = 1280                # bf16 [128, 512]: wdyn | wfold (rows 33: zero)
O_WV1 = 1536                # bf16 [128, 512]: 0.5*Wvd1 rows 0:128 | 128:256
O_IDB = 1792                # bf16 identity [128, 128]
O_CW = 1856                 # fp32 [128, 5]
O_CB = 1861                 # fp32 [128, 1]
WPACK = 1862

F32 = mybir.dt.float32
BF16 = mybir.dt.bfloat16
MULT = mybir.AluOpType.mult
ADD = mybir.AluOpType.add
IDENT = mybir.ActivationFunctionType.Identity
EXP = mybir.ActivationFunctionType.Exp

_NC_CACHE = None
LAST_RESULTS = None
TRACE = False


def _patched_drain_and_barrier(self, tick_clock, wait_clock):
    # The stock kernel-tail drain puts every processor's final-tick wait on a
    # single Drain instruction; this walrus build rejects multi-wait
    # instructions, so spread the waits over a chain of drains instead.
    import bass_rust as _br
    from concourse.vector_clock import ScopedClock

    nc = self.nc
    drain_inst = nc.sync.drain()
    wait_clock.add_sem_waits(
        drain_inst.ins, ScopedClock({None: tick_clock.global_clock})
    )
    si = drain_inst.ins.sync_info
    if si is not None and len(si.on_wait) > 1:
        waits = list(si.on_wait)
        drain_inst.ins.sync_info = _br.SyncInfo(
            on_wait=[waits[0]], on_update=list(si.on_update)
        )
        for w in waits[1:]:
            d2 = nc.sync.drain()
            d2.ins.sync_info = _br.SyncInfo(on_wait=[w], on_update=[])
    nc.all_engine_barrier()
    assert self.sems is not None
    popped = nc._tile_sem_poison_stack.pop()
    assert popped is self._sem_poison
    nc.clear_and_free_semaphores(list(self.sems.allocated().values()))
    nc.all_engine_barrier()


tile.TileContext._drain_and_barrier = _patched_drain_and_barrier


def _build_nc():
    # Bacc (not plain Bass): its finalize() runs move_matmul_waits_to_ldweights
    # and generate_event_semaphores, which split multi-wait instructions into
    # EventSemaphore preludes — this build's ISA allows one wait per
    # instruction (two on EventSemaphore).
    nc = bacc.Bacc()
    actb_d = nc.dram_tensor("actb", [P, 2 * R], BF16, kind="ExternalInput")
    wpack_d = nc.dram_tensor("wpack", [P, WPACK], F32, kind="ExternalInput")
    out_d = nc.dram_tensor("out", [R, E], F32, kind="ExternalOutput")
    actb3 = actb_d[:, :].rearrange("p (two r) -> p two r", two=2)

    with tile.TileContext(nc) as tc, ExitStack() as ctx:
        consts = ctx.enter_context(tc.tile_pool(name="consts", bufs=1))
        ps1 = ctx.enter_context(tc.tile_pool(name="ps1", bufs=1, space="PSUM"))
        ps2 = ctx.enter_context(tc.tile_pool(name="ps2", bufs=1, space="PSUM"))
        ps3 = ctx.enter_context(tc.tile_pool(name="ps3", bufs=2, space="PSUM"))
        psD = ctx.enter_context(tc.tile_pool(name="psD", bufs=3, space="PSUM"))
        psT = ctx.enter_context(tc.tile_pool(name="psT", bufs=1, space="PSUM"))
        pc = ctx.enter_context(tc.tile_pool(name="pc", bufs=16))
        ps = ctx.enter_context(tc.tile_pool(name="ps", bufs=16))
        pz = ctx.enter_context(tc.tile_pool(name="pz", bufs=16))
        pzT = ctx.enter_context(tc.tile_pool(name="pzT", bufs=16))
        po = ctx.enter_context(tc.tile_pool(name="po", bufs=16))
        psm = ctx.enter_context(tc.tile_pool(name="psm", bufs=16))

        wp = consts.tile([P, WPACK], F32)
        ab = consts.tile([P, 2, R], BF16)

        # first-needed inputs on SP (HWDGE, ~565ns issue each); the rest on
        # the otherwise-idle Pool Q7 (SWDGE, ~1us generation each, parallel)
        nc.sync.dma_start(wp[:, 0:512], wpack_d[:, 0:512])
        nc.sync.dma_start(ab[:, :, 0:2 * P], actb3[:, :, 0:2 * P])
        nc.gpsimd.dma_start(wp[:, O_WDF:WPACK], wpack_d[:, O_WDF:WPACK])
        nc.gpsimd.dma_start(wp[:, 512:1280], wpack_d[:, 512:1280])
        QC = 2 * P  # actb DMA chunk: 2 tiles
        for q in range(1, R // QC):
            nc.gpsimd.dma_start(
                ab[:, :, q * QC : (q + 1) * QC], actb3[:, :, q * QC : (q + 1) * QC]
            )

        wpb = wp[:].bitcast(BF16)
        wgA1 = wpb[:, 2 * O_WGA1 : 2 * O_WGA1 + 512]
        wgA2 = wpb[0:K2, 2 * O_WGA2 : 2 * O_WGA2 + 512]
        wgB1 = wpb[:, 2 * O_WGB1 : 2 * O_WGB1 + 768]
        wgB2 = wpb[0:K2, 2 * O_WGB2 : 2 * O_WGB2 + 768]
        wdf = wpb[:, 2 * O_WDF : 2 * O_WDF + 512]
        wv1a = wpb[:, 2 * O_WV1 : 2 * O_WV1 + E]
        wv1b = wpb[:, 2 * O_WV1 + E : 2 * O_WV1 + 2 * E]
        identb = wpb[:, 2 * O_IDB : 2 * O_IDB + P]
        cw = wp[:, O_CW : O_CW + KW]
        cb = wp[:, O_CB : O_CB + 1]

        # warmup: each engine observes the weight DMAs once so later compute
        # instructions rarely need a second (split) sem wait for them
        nc.tensor.ldweights(wgA1[0:1, 0:2])
        nc.tensor.ldweights(wgB1[0:1, 0:2])
        nc.tensor.ldweights(wdf[0:1, 0:2])
        at = psm.tile([P, 1], F32, tag="wm")
        nc.scalar.copy(at[:], cb)
        dt_ = psm.tile([P, 1], F32, tag="wm2")
        nc.vector.tensor_copy(dt_[:], cb)

        # pipeline state: st1[i] after stage 1, st2[i] after stage 2a
        st1 = {}
        st2 = {}

        for it in range(NTILES + 2):
            # ---- stage 1 PE (tile it): G + dyn/fold matmuls -------------
            if it < NTILES:
                i = it
                rows = slice(i * P, (i + 1) * P)
                b0 = ab[:, 0, rows]
                b1 = ab[0:K2, 1, rows]

                T1 = ps1.tile([P, 512], F32, tag="T1")
                T2 = ps2.tile([P, 512], F32, tag="T2")
                T3 = ps3.tile([P, 256], F32, tag="T3")
                pdf = psD.tile([P, 512], F32, tag="pdf")

                nc.tensor.matmul(T1[:, 0:512], b0, wgA1,
                                 start=True, stop=False, skip_group_check=True)
                nc.tensor.matmul(T1[:, 0:512], b1, wgA2,
                                 start=False, stop=True, skip_group_check=True)
                nc.tensor.matmul(T2[:, 0:512], b0, wgB1[:, 0:512],
                                 start=True, stop=False, skip_group_check=True)
                nc.tensor.matmul(T2[:, 0:512], b1, wgB2[:, 0:512],
                                 start=False, stop=True, skip_group_check=True)
                nc.tensor.matmul(T3[:, 0:256], b0, wgB1[:, 512:768],
                                 start=True, stop=False, skip_group_check=True)
                nc.tensor.matmul(T3[:, 0:256], b1, wgB2[:, 512:768],
                                 start=False, stop=True, skip_group_check=True)
                nc.tensor.matmul(pdf[:, 0:512], b0, wdf,
                                 start=True, stop=True, skip_group_check=True)

            # ---- stage 2b PE (tile it-2): zT @ 0.5*Wvd1 into pdf --------
            if it >= 2:
                zTs2, pdf2 = st2[it - 2]
                nc.tensor.matmul(pdf2[:, 256:512], zTs2[:, 0, :], wv1a,
                                 start=False, stop=False, skip_group_check=True)
                nc.tensor.matmul(pdf2[:, 256:512], zTs2[:, 1, :], wv1b,
                                 start=False, stop=True, skip_group_check=True)

            # ---- stage 1 ACT/DVE (tile it): conv combine + relu ---------
            if it < NTILES:
                c0 = pc.tile([P, E], BF16, tag="c0")
                nc.scalar.activation(c0[:], T1[:, 0:256], func=IDENT,
                                     scale=cw[:, 0:1], bias=cb)
                c1 = pc.tile([P, E], BF16, tag="c1")
                nc.scalar.activation(c1[:], T1[:, 256:512], func=IDENT,
                                     scale=cw[:, 1:2])
                s1 = ps.tile([P, E], BF16, tag="s1")
                nc.vector.tensor_add(s1[:], c0[:], c1[:])
                s2 = ps.tile([P, E], BF16, tag="s2")
                nc.vector.scalar_tensor_tensor(s2[:], T2[:, 0:256], cw[:, 2:3],
                                               s1[:], op0=MULT, op1=ADD)
                s3 = ps.tile([P, E], BF16, tag="s3")
                nc.vector.scalar_tensor_tensor(s3[:], T2[:, 256:512], cw[:, 3:4],
                                               s2[:], op0=MULT, op1=ADD)
                s4 = ps.tile([P, E], BF16, tag="s4")
                nc.vector.scalar_tensor_tensor(s4[:], T3[:, 0:256], cw[:, 4:5],
                                               s3[:], op0=MULT, op1=ADD)
                rel = ps.tile([P, E], BF16, tag="rel")
                nc.vector.tensor_scalar_max(rel[:], s4[:], 0.0)
                st1[i] = (rel, pdf)

            # ---- stage 2a (tile it-1): softmax + gate + transpose -------
            if 1 <= it <= NTILES:
                j = it - 1
                relj, pdfj = st1.pop(j)
                exm = pc.tile([P, E], BF16, tag="exm")
                ssum = psm.tile([P, 1], F32, tag="ssum")
                nc.scalar.activation(exm[:], relj[:], func=EXP, accum_out=ssum[:])
                sinv = psm.tile([P, 1], F32, tag="sinv")
                nc.vector.reciprocal(sinv[:], ssum[:])
                z = pz.tile([P, E], BF16, tag="z")
                nc.vector.scalar_tensor_tensor(z[:], exm[:], sinv[:],
                                               pdfj[:, 0:256], op0=MULT, op1=MULT)
                ptz = psT.tile([P, 2, P], BF16, tag="ptz")
                nc.tensor.transpose(ptz[:, 0, :], z[:, 0:128], identb)
                nc.tensor.transpose(ptz[:, 1, :], z[:, 128:256], identb)
                zTs = pzT.tile([P, 2, P], BF16, tag="zTs", name=f"zTs{j}")
                nc.vector.tensor_copy(zTs[:], ptz[:])
                st2[j] = (zTs, pdfj)

            # ---- stage 2c (tile it-2): out drain + DMA ------------------
            if it >= 2:
                zTs2, pdf2 = st2.pop(it - 2)
                ob = po.tile([P, E], F32, tag="ob", name=f"ob{it - 2}")
                nc.scalar.copy(ob[:], pdf2[:, 256:512])
                nc.gpsimd.dma_start(
                    out_d[(it - 2) * P : (it - 1) * P, :], ob[:]
                )

    nc.finalize()
    return nc


def _host_prep(x, human, nature, perm, Wv, bv, Wd, bd, Wh, bh, Wn, bn,
               conv_w, conv_b, Wvd, bvd):
    f = np.float32
    bf = ml_dtypes.bfloat16
    x = np.asarray(x, f)
    human = np.asarray(human, f)
    nature = np.asarray(nature, f)
    Wv = np.asarray(Wv, f); bv = np.asarray(bv, f)
    Wd = np.asarray(Wd, f); bd = np.asarray(bd, f)
    Wh = np.asarray(Wh, f); bh = np.asarray(bh, f)
    Wn = np.asarray(Wn, f); bn = np.asarray(bn, f)
    conv_w = np.asarray(conv_w, f)
    conv_b = np.asarray(conv_b, f)
    Wvd = np.asarray(Wvd, f); bvd = np.asarray(bvd, f)
    perm = np.asarray(perm).astype(np.int64)

    Wvd1 = Wvd[:E, :]
    Wvd2 = Wvd[E:, :]

    acts = np.concatenate(
        [
            x.reshape(B * T, XS),
            np.ones((B * T, 1), f),
            human.reshape(B * T, HT),
            nature.reshape(B * T, NT_),
        ],
        axis=1,
    )
    actsT = np.ascontiguousarray(acts.T)  # [193, B*T]
    actb = np.zeros((P, 2, B * T), bf)
    actb[:, 0, :] = actsT[0:128]
    actb[0:K2, 1, :] = actsT[128:AK]

    wpack = np.zeros((P, WPACK), f)
    wpv = wpack.view(bf)  # bf16 alias [128, 2*WPACK]

    # dyn | folded-linear weights (rows 33:128 zero so the matmul can use
    # the full 128-row stationary block)
    wdf = np.zeros((128, 512), f)
    wdf[0:DS, 0:E] = Wd
    wdf[32, 0:E] = bd
    wdf[0:DS, E:512] = 0.5 * (Wd @ Wvd1)
    wdf[31, E:512] = Wv[0] @ Wvd2
    wdf[32, E:512] = 0.5 * (bd @ Wvd1) + bv @ Wvd2 + bvd
    wpv[:, 2 * O_WDF : 2 * O_WDF + 512] = wdf.astype(bf)

    # gathered conv weights (bf16)
    ainv = np.argsort(perm)
    Wg = np.zeros((AK, NG), f)
    for k in range(KW):
        pos = ainv[:E] + k - 3
        for j in range(E):
            pj = pos[j]
            if 0 <= pj < C:
                c = perm[pj]
                col = k * E + j
                if c < E:
                    Wg[0:DS, col] = Wd[:, c]
                    Wg[32, col] = bd[c]
                elif c < 2 * E:
                    Wg[33:113, col] = Wh[:, c - E]
                    Wg[32, col] = bh[c - E]
                else:
                    Wg[113:193, col] = Wn[:, c - 2 * E]
                    Wg[32, col] = bn[c - 2 * E]
    wpv[:, 2 * O_WGA1 : 2 * O_WGA1 + 512] = Wg[0:128, 0:512].astype(bf)
    wpv[0:K2, 2 * O_WGA2 : 2 * O_WGA2 + 512] = Wg[128:AK, 0:512].astype(bf)
    wpv[:, 2 * O_WGB1 : 2 * O_WGB1 + 768] = Wg[0:128, 512:1280].astype(bf)
    wpv[0:K2, 2 * O_WGB2 : 2 * O_WGB2 + 768] = Wg[128:AK, 512:1280].astype(bf)

    # 0.5 * Wvd1 (bf16), split into two K-chunks
    wv1 = (0.5 * Wvd1).astype(bf)
    wpv[:, 2 * O_WV1 : 2 * O_WV1 + E] = wv1[0:128]
    wpv[:, 2 * O_WV1 + E : 2 * O_WV1 + 2 * E] = wv1[128:256]

    wpv[:, 2 * O_IDB : 2 * O_IDB + P] = np.eye(P, dtype=bf)
    wpack[:, O_CW : O_CW + KW] = np.tile(conv_w[:, 0, :], (2, 1))
    wpack[:, O_CB] = np.tile(conv_b, 2)
    return actb, wpack


def kernel(**inputs):
    global _NC_CACHE, LAST_RESULTS
    actb, wpack = _host_prep(**inputs)

    if _NC_CACHE is None:
        _NC_CACHE = _build_nc()
    nc = _NC_CACHE

    in_maps = []
    for ci in range(NCORES):
        sb = np.ascontiguousarray(actb[:, :, ci * R : (ci + 1) * R]).reshape(
            P, 2 * R
        )
        in_maps.append({"actb": sb, "wpack": wpack})

    res = run_bass_kernel_spmd(nc, in_maps, core_ids=list(range(NCORES)), trace=TRACE)
    LAST_RESULTS = res

    out = np.empty((B, T, E), np.float32)
    for ci in range(NCORES):
        out[ci * BPC : (ci + 1) * BPC] = res.results[ci]["out"].reshape(BPC, T, E)
    return out
